# revision 62
# baseline (speedup 1.0000x reference)
"""Masked dot-product attention (B=8, Lq=Lk=2048, D=64) on 8 Trainium2 NeuronCores.

Strategy
--------
Only keys k < valid_len[b] contribute (exp(-1e6) underflows to exactly 0), and
scores are ~N(0,1) so softmax needs no max-subtraction; unnormalized partial
sums over key-chunks are purely additive.  We therefore split work at
(batch, 128-key-chunk) granularity and load-balance those units across the 8
cores, combining partials on the host.

Per work unit (batch b, key chunk c), a core computes (layouts transposed so
no on-chip transposes are ever needed; all matmul operands bf16, fp32 PSUM):
    S^T[k, q] = K_c^T Q^T          (PE, bf16, contraction d=64)
    E = exp(S^T/8 + mask_bias)     (h0 on ACT: table exp; h1 on DVE:
                                    Schraudolph int16 fast-exp written
                                    straight into a bf16 tile via bitcast)
    O^T[d', q] += V'_c^T E         (PE, bf16, contraction k=128)
where V' = [V_c | 1] so row 64 of O^T accumulates the softmax denominator.

The device program is a software pipeline over the global unit list: at step
t the PE runs S_t then AV_{t-1}, while ACT does exp_t(h0) and DVE does
exp_t(h1) concurrently - sized so the PE (8 x 512-col matmuls / 1.7us per
unit at 2.4 GHz) is the binding engine, not the activations.  Slot psO
casts split ACT(h0)/DVE(h1); outputs stream per-slot on two DMA queues.

Inputs are packed on the host into DMA-friendly layouts (one contiguous
free-dim row per SBUF partition) and streamed over three queues (sync,
scalar/ACT, gpsimd) with the first unit's working set prioritized; a short
burst of dummy bf16 matmuls bridges the DMA head so the PE clock is warm
when real work begins.
"""

import sys
import math

sys.path.insert(0, "/opt/trn_rl_repo")

import numpy as np
import ml_dtypes

import concourse.bass as bass
import concourse.bacc as bacc
import concourse.mybir as mybir
import concourse.tile as tile
from concourse.bass_utils import run_bass_kernel_spmd

F32 = mybir.dt.float32
BF16 = mybir.dt.bfloat16
I16 = mybir.dt.int16

B, L, D = 8, 2048, 64
NCORES = 8
CHUNK = 128          # key rows per work unit
NEG = -1e6
SCALE = 1.0 / 8.0    # 1/sqrt(64)
QH = 1024            # q processed in halves for PSUM budget
N_WARM_PRE = 7       # dummy PE matmuls before the first real matmul: bridge
                     # the DMA head and ramp the clock to 2.4GHz (cold PE
                     # runs ~2x slower and a gap-ridden pipeline never ramps)
N_WARM_POST = 0      # more dummies between unit 0's S and unit 1
N_WARM_POST1 = 0     # one more inside step 1, before unit 0's AV
VPW = D + 6          # V'|pad slots per unit, bf16 (padding VPW to 128 to get
                     # Fast Weight Load on the AV matmuls measured slightly
                     # WORSE end-to-end: +133KB input DMA beats the LDW win)

# Schraudolph fast-exp on the DVE engine: exp(s*SCALE + mb) is approximated
# by i16 = round(s*A + B) reinterpreted as bf16 bits (max rel err 3.3%, which
# averages out over the many valid keys of any offloaded unit).  Masked rows
# get B ~ -3e7 so the int16 convert saturates to -32768 = bf16 -0.0.
SCHRAUD_A = float(SCALE * np.log2(np.e) * 128)   # 23.083...
SCHRAUD_SIGMA = 32.0   # tuned numerically against the f64 reference
SCHRAUD_B0 = 16256.0 - SCHRAUD_SIGMA


# --------------------------------------------------------------------------
# host-side scheduling: assign (batch, chunk) units to (core, slot) bins
# --------------------------------------------------------------------------

def _greedy_assign(chunks, caps):
    """Assign each batch's chunks to bins of 8 cores x caps; each bin holds
    chunks of a single batch.  Returns {(core, slot): (batch, [chunk_ids])}
    or None if infeasible."""
    bins = []  # (cap, core, slot)
    for core in range(NCORES):
        for s, c in enumerate(caps):
            bins.append([c, core, s])
    order = sorted(range(len(chunks)), key=lambda b: -chunks[b])
    free = sorted(bins, key=lambda x: -x[0])
    assign = {}
    for b in order:
        rem = chunks[b]
        next_chunk = 0
        while rem > 0:
            if not free:
                return None
            pick = None
            for i in range(len(free) - 1, -1, -1):
                if free[i][0] >= rem:
                    pick = i
                    break
            if pick is None:
                pick = 0
            cap, core, s = free.pop(pick)
            take = min(cap, rem)
            assign[(core, s)] = (b, list(range(next_chunk, next_chunk + take)))
            next_chunk += take
            rem -= take
    return assign


def _schedule(chunks):
    """Pick slot capacities (shared program structure) + assignment."""
    total = sum(chunks)
    lo = max(1, math.ceil(total / NCORES))
    for U in range(lo, 17):
        caps_opts = []
        for c0 in range(U, 0, -1):
            for c1 in range(min(c0, U - c0), -1, -1):
                c2 = U - c0 - c1
                if c2 < 0 or c2 > c1:
                    continue
                caps = tuple(c for c in (c0, c1, c2) if c > 0)
                caps_opts.append(caps)
        caps_opts.sort(key=lambda cs: (len(cs), max(cs)))
        for caps in caps_opts:
            asg = _greedy_assign(chunks, caps)
            if asg is not None:
                return caps, asg
    caps = (16,)
    asg = {(b, 0): (b, list(range(chunks[b]))) for b in range(B)}
    return caps, asg


# --------------------------------------------------------------------------
# device program (one NEFF shared by all 8 cores; structure = caps)
# --------------------------------------------------------------------------

def _build_program(caps):
    S = len(caps)
    U = sum(caps)
    nc = bacc.Bacc("TRN2", target_bir_lowering=False)

    # q's two halves live on the two partition halves so DMA uses all 16
    # SBUF ports; K^T is duplicated so either row-group can contract with it.
    # All DRAM layouts match the SBUF layouts so each DMA moves one long
    # contiguous run per partition (fewest descriptors).
    qts_d = nc.dram_tensor("qts", [S, 2 * D, QH], BF16, kind="ExternalInput")
    ktp_d = nc.dram_tensor("ktp", [2 * D, U, CHUNK], BF16, kind="ExternalInput")
    vp_d = nc.dram_tensor("vp", [CHUNK, U, VPW], BF16, kind="ExternalInput")
    mbb_d = nc.dram_tensor("mbb", [CHUNK, 2, U], F32, kind="ExternalInput")
    out_d = nc.dram_tensor("out", [S, D + 1, L], BF16, kind="ExternalOutput")

    # global unit order with slot bookkeeping
    slot_of = []
    idx_in_slot = []
    for s, c in enumerate(caps):
        for i in range(c):
            slot_of.append(s)
            idx_in_slot.append(i)

    with tile.TileContext(nc) as tc:
        with (
            tc.tile_pool(name="const", bufs=1) as const,
            tc.tile_pool(name="psS", bufs=2, space="PSUM") as psS_pool,
            tc.tile_pool(name="psO", bufs=2, space="PSUM") as psO_pool,
            tc.tile_pool(name="epool", bufs=3) as epool,
            tc.tile_pool(name="stage", bufs=3) as stage_pool,
        ):
            qts_sb = const.tile([2 * D, S, QH], BF16, tag="qts")
            ktp_sb = const.tile([2 * D, U, CHUNK], BF16, tag="ktp")
            vp_sb = const.tile([CHUNK, U, VPW], BF16, tag="vp")
            mbb_sb = const.tile([CHUNK, 2, U], F32, tag="mbb")
            warm_sb = const.tile([128, 512], BF16, tag="warm")

            # PE warm-up: dummy bf16 matmuls with no DMA dependency keep the
            # PE busy while inputs stream in, so the clock is ramped when the
            # first real matmul issues.  The memset is the very first gpsimd
            # instruction so warmups start as early as possible.
            nc.gpsimd.memset(warm_sb[:], 0.0)

            def emit_warm(n):
                for wi in range(n):
                    wps = psO_pool.tile([128, 512], F32, tag="psO")
                    nc.tensor.matmul(wps[:], warm_sb[:, :128], warm_sb[:], start=True, stop=True)

            emit_warm(N_WARM_PRE)

            # ---- input DMA: two HWDGE rings, ring-FIFO = priority order.
            # The first units' working set heads each ring; bulk follows and
            # naturally waits its turn, so the critical bytes never contend
            # for HBM bandwidth with the bulk.
            u3 = min(3, U)
            # scalar ring: K^T unit 0 (smallest, gates the first matmul),
            # then slot-0 Q's second column-half (splitting the critical Q
            # across BOTH rings nearly halves its arrival time — each ring
            # only gets ~half of HBM bandwidth), K^T units 1-2, V' units
            # 0-2, then the bulk
            nc.scalar.dma_start(ktp_sb[:, 0:1, :], ktp_d[:, 0:1, :])
            nc.scalar.dma_start(qts_sb[:, 0, 512:QH], qts_d[0, :, 512:QH])
            if u3 > 1:
                nc.scalar.dma_start(ktp_sb[:, 1:u3, :], ktp_d[:, 1:u3, :])
            nc.scalar.dma_start(vp_sb[:, 0:u3, :], vp_d[:, 0:u3, :])
            if U > u3:
                nc.scalar.dma_start(ktp_sb[:, u3:U, :], ktp_d[:, u3:U, :])
                nc.scalar.dma_start(vp_sb[:, u3:U, :], vp_d[:, u3:U, :])
            # sync ring: mask biases (small, hides under the warmup window),
            # then slot-0 Q's first column-half, then remaining Q slots
            nc.sync.dma_start(mbb_sb[:], mbb_d[:, :, :])
            nc.sync.dma_start(qts_sb[:, 0, 0:512], qts_d[0, :, 0:512])
            for s in range(1, S):
                nc.sync.dma_start(qts_sb[:, s, :], qts_d[s, :, :])

            # ---- software pipeline over the global unit list ----
            def emit_S(t):
                s = slot_of[t]
                psS_h = [
                    psS_pool.tile([CHUNK, QH], F32, tag="psS", name=f"psS_{t}_{hh}")
                    for hh in range(2)
                ]
                for h in range(2):
                    rows = slice(h * D, (h + 1) * D)
                    for j in range(QH // 512):
                        nc.tensor.matmul(
                            psS_h[h][:, j * 512 : (j + 1) * 512],
                            ktp_sb[rows, t, :],
                            qts_sb[rows, s, j * 512 : (j + 1) * 512],
                            start=True,
                            stop=True,
                        )
                return psS_h

            def emit_exp(t, psS_h):
                # h0 on ACT (table exp); h1 Schraudolph int16 fast-exp into
                # bf16 bits — on DVE, except act_schraud units where the
                # same fast-exp runs on ACT (Identity activation) to balance
                # the two engines' totals.  The exp METHOD stays consistent
                # per (batch, half): mixing table-exp and Schraudolph within
                # one softmax column breaks the bias cancellation in the
                # numerator/denominator ratio (measured 3.2e-2 vs 1.3e-2).
                e_h = []
                for h in range(2):
                    e = epool.tile([CHUNK, QH], BF16, tag=f"e{h}")
                    if h == 0:
                        nc.scalar.activation(
                            e[:],
                            psS_h[h][:],
                            mybir.ActivationFunctionType.Exp,
                            bias=mbb_sb[:, 0, t : t + 1],
                            scale=SCALE,
                        )
                    elif t in act_schraud:
                        nc.scalar.activation(
                            e[:].bitcast(I16),
                            psS_h[h][:],
                            mybir.ActivationFunctionType.Identity,
                            bias=mbb_sb[:, 1, t : t + 1],
                            scale=SCHRAUD_A,
                        )
                    else:
                        nc.vector.tensor_scalar(
                            e[:].bitcast(I16),
                            psS_h[h][:],
                            SCHRAUD_A,
                            mbb_sb[:, 1, t : t + 1],
                            mybir.AluOpType.mult,
                            mybir.AluOpType.add,
                        )
                    e_h.append(e)
                return e_h

            def emit_AV(t, psO_h, e_h):
                s = slot_of[t]
                first = idx_in_slot[t] == 0
                last = idx_in_slot[t] == caps[s] - 1
                for h in range(2):
                    for j in range(QH // 512):
                        nc.tensor.matmul(
                            psO_h[h][:, j * 512 : (j + 1) * 512],
                            vp_sb[:, t, 0 : D + 1],
                            e_h[h][:, j * 512 : (j + 1) * 512],
                            start=first,
                            stop=last,
                        )

            def emit_out(s, psO_h, last=False):
                # cast h0 on ACT, h1 on DVE, stream to DRAM on the two hwdge
                # rings.  Only the final slot casts in 512-col pieces
                # (subtile deps let piece j0 start as soon as the final AV j0
                # matmul lands) to shorten the kernel tail; piece-casting
                # every slot bursts the exp engines at slot boundaries and
                # stalls the PE.
                st0 = stage_pool.tile([D + 1, QH], BF16, tag="st0")
                st1 = stage_pool.tile([D + 1, QH], BF16, tag="st1")
                npc = QH // 512 if last else 1
                w = QH // npc
                for j in range(npc):
                    c = slice(j * w, (j + 1) * w)
                    nc.scalar.activation(st0[:, c], psO_h[0][:, c], mybir.ActivationFunctionType.Copy)
                    nc.sync.dma_start(out_d[s, :, j * w : (j + 1) * w], st0[:, c])
                    nc.vector.tensor_copy(st1[:, c], psO_h[1][:, c])
                    nc.scalar.dma_start(out_d[s, :, QH + j * w : QH + (j + 1) * w], st1[:, c])

            # units whose h1 Schraudolph runs on ACT instead of DVE (engine
            # load balance only — same method, so columns stay consistent)
            act_schraud = set()  # ACT Identity triggers a table switch (~3us)
                                 # mid-kernel — measured slower; keep all
                                 # Schraudolph on DVE

            slot_psO = {}  # slot -> psO tile pair, allocated at first AV

            def do_AV(pt, p_eh):
                ps = slot_of[pt]
                if idx_in_slot[pt] == 0:
                    slot_psO[ps] = [
                        psO_pool.tile([D + 1, QH], F32, tag="psO", name=f"psO_{ps}_{hh}")
                        for hh in range(2)
                    ]
                emit_AV(pt, slot_psO[ps], p_eh)
                if idx_in_slot[pt] == caps[ps] - 1:
                    emit_out(ps, slot_psO[ps], last=(pt == U - 1))

            pend = None  # (t, e_h) awaiting AV
            for t in range(U):
                psS_h = emit_S(t)
                if t == 1:
                    emit_warm(N_WARM_POST1)
                if pend is not None:
                    pt, p_eh = pend
                    do_AV(pt, p_eh)
                e_h = emit_exp(t, psS_h)
                pend = (t, e_h)
                if t == 0:
                    # fill the pipeline-fill bubble (unit 1's S waits on
                    # unit 0's exp) with more clock-warming dummies
                    emit_warm(N_WARM_POST)
            pt, p_eh = pend
            do_AV(pt, p_eh)
    nc.compile()
    return nc


# --------------------------------------------------------------------------
# host packing + gather
# --------------------------------------------------------------------------

def _pack_inputs(Q, K, V, valid_len, caps, asg):
    S = len(caps)
    U = sum(caps)
    slot_u0 = np.cumsum([0] + list(caps))[:-1]

    QT = np.ascontiguousarray(Q.transpose(0, 2, 1))  # [B, D, L]
    KT = np.ascontiguousarray(K.transpose(0, 2, 1))  # [B, D, L]

    bf16 = ml_dtypes.bfloat16
    in_maps = []
    for core in range(NCORES):
        qts = np.zeros((S, 2 * D, QH), bf16)
        ktp = np.zeros((2 * D, U, CHUNK), bf16)
        vp = np.zeros((CHUNK, U, VPW), bf16)
        # default biases for unassigned units: fully masked
        mbb = np.zeros((CHUNK, U, 2), np.float32)
        mbb[:, :, 0] = NEG
        mbb[:, :, 1] = -3.0e7
        for s in range(S):
            ent = asg.get((core, s))
            if ent is None:
                continue
            b, chunk_ids = ent
            qts[s, :D] = QT[b][:, :QH]
            qts[s, D:] = QT[b][:, QH:]
            for i, c in enumerate(chunk_ids):
                u = slot_u0[s] + i
                k0 = c * CHUNK
                ktp[:D, u] = KT[b][:, k0 : k0 + CHUNK]
                ktp[D:, u] = KT[b][:, k0 : k0 + CHUNK]
                vp[:, u, :D] = V[b][k0 : k0 + CHUNK]
                nvalid = int(min(max(valid_len[b] - k0, 0), CHUNK))
                vp[:nvalid, u, D] = 1.0
                mbb[:nvalid, u, 0] = 0.0
                mbb[:nvalid, u, 1] = SCHRAUD_B0
        mbb2 = np.ascontiguousarray(mbb.transpose(0, 2, 1))  # [CHUNK, 2, U]
        in_maps.append({"qts": qts, "ktp": ktp, "vp": vp, "mbb": mbb2})
    return in_maps


def _gather(results, caps, asg):
    acc = np.zeros((B, D + 1, L), np.float64)
    for core in range(NCORES):
        out = results[core]["out"]  # [S, D+1, L]
        for s in range(len(caps)):
            ent = asg.get((core, s))
            if ent is None:
                continue
            b, _ = ent
            acc[b] += np.asarray(out[s], dtype=np.float64)
    out = acc[:, :D, :] / acc[:, D : D + 1, :]
    return np.ascontiguousarray(out.transpose(0, 2, 1)).astype(np.float32)


_PROGRAM_CACHE = {}


def kernel(Q, K, V, valid_len, **kw):
    Q = np.asarray(Q, dtype=np.float32)
    K = np.asarray(K, dtype=np.float32)
    V = np.asarray(V, dtype=np.float32)
    vl = np.asarray(valid_len).astype(np.int64)

    chunks = [int(math.ceil(max(int(v), 1) / CHUNK)) for v in vl]
    caps, asg = _schedule(chunks)

    if caps not in _PROGRAM_CACHE:
        _PROGRAM_CACHE[caps] = _build_program(caps)
    nc = _PROGRAM_CACHE[caps]

    in_maps = _pack_inputs(Q, K, V, vl, caps, asg)
    res = run_bass_kernel_spmd(nc, in_maps, core_ids=list(range(NCORES)))
    return _gather(res.results, caps, asg)


# revision 66
# speedup vs baseline: 1.1072x; 1.1072x over previous
"""Masked dot-product attention (B=8, Lq=Lk=2048, D=64) on 8 Trainium2 NeuronCores.

Strategy
--------
Only keys k < valid_len[b] contribute (exp(-1e6) underflows to exactly 0), and
scores are ~N(0,1) so softmax needs no max-subtraction; unnormalized partial
sums over key-chunks are purely additive.  We therefore split work at
(batch, 128-key-chunk) granularity and load-balance those units across the 8
cores, combining partials on the host.

Per work unit (batch b, key chunk c), a core computes (layouts transposed so
no on-chip transposes are ever needed; all matmul operands bf16, fp32 PSUM):
    S^T[k, q] = K_c^T Q^T          (PE, bf16, contraction d=64)
    E = exp(S^T/8 + mask_bias)     (h0 on ACT: table exp; h1 on DVE:
                                    Schraudolph int16 fast-exp written
                                    straight into a bf16 tile via bitcast)
    O^T[d', q] += V'_c^T E         (PE, bf16, contraction k=128)
where V' = [V_c | 1] so row 64 of O^T accumulates the softmax denominator.

The device program is a software pipeline over the global unit list: at step
t the PE runs S_t then AV_{t-1}, while ACT does exp_t(h0) and DVE does
exp_t(h1) concurrently - sized so the PE (8 x 512-col matmuls / 1.7us per
unit at 2.4 GHz) is the binding engine, not the activations.  Slot psO
casts split ACT(h0)/DVE(h1); outputs stream per-slot on two DMA queues.

Inputs are packed on the host into DMA-friendly layouts (one contiguous
free-dim row per SBUF partition) and streamed over three queues (sync,
scalar/ACT, gpsimd) with the first unit's working set prioritized; a short
burst of dummy bf16 matmuls bridges the DMA head so the PE clock is warm
when real work begins.
"""

import sys
import math

sys.path.insert(0, "/opt/trn_rl_repo")

import numpy as np
import ml_dtypes

import concourse.bass as bass
import concourse.bacc as bacc
import concourse.mybir as mybir
import concourse.tile as tile
from concourse.bass_utils import run_bass_kernel_spmd

F32 = mybir.dt.float32
BF16 = mybir.dt.bfloat16
I16 = mybir.dt.int16

B, L, D = 8, 2048, 64
NCORES = 8
CHUNK = 128          # key rows per work unit
NEG = -1e6
SCALE = 1.0 / 8.0    # 1/sqrt(64)
QH = 1024            # q processed in halves for PSUM budget
N_WARM_PRE = 11      # dummy PE matmuls before the first real matmul: bridge
                     # the DMA head and ramp the clock to 2.4GHz (cold PE
                     # runs ~2x slower and a gap-ridden pipeline never ramps)
N_WARM_POST = 0      # more dummies between unit 0's S and unit 1
N_WARM_POST1 = 0     # one more inside step 1, before unit 0's AV
VPW = D + 6          # V'|pad slots per unit, bf16 (padding VPW to 128 to get
                     # Fast Weight Load on the AV matmuls measured slightly
                     # WORSE end-to-end: +133KB input DMA beats the LDW win)

# Schraudolph fast-exp on the DVE engine: exp(s*SCALE + mb) is approximated
# by i16 = round(s*A + B) reinterpreted as bf16 bits (max rel err 3.3%, which
# averages out over the many valid keys of any offloaded unit).  Masked rows
# get B ~ -3e7 so the int16 convert saturates to -32768 = bf16 -0.0.
SCHRAUD_A = float(SCALE * np.log2(np.e) * 128)   # 23.083...
SCHRAUD_SIGMA = 32.0   # tuned numerically against the f64 reference
SCHRAUD_B0 = 16256.0 - SCHRAUD_SIGMA


# --------------------------------------------------------------------------
# host-side scheduling: assign (batch, chunk) units to (core, slot) bins
# --------------------------------------------------------------------------

def _greedy_assign(chunks, caps):
    """Assign each batch's chunks to bins of 8 cores x caps; each bin holds
    chunks of a single batch.  Returns {(core, slot): (batch, [chunk_ids])}
    or None if infeasible."""
    bins = []  # (cap, core, slot)
    for core in range(NCORES):
        for s, c in enumerate(caps):
            bins.append([c, core, s])
    order = sorted(range(len(chunks)), key=lambda b: -chunks[b])
    free = sorted(bins, key=lambda x: -x[0])
    assign = {}
    for b in order:
        rem = chunks[b]
        next_chunk = 0
        while rem > 0:
            if not free:
                return None
            pick = None
            for i in range(len(free) - 1, -1, -1):
                if free[i][0] >= rem:
                    pick = i
                    break
            if pick is None:
                pick = 0
            cap, core, s = free.pop(pick)
            take = min(cap, rem)
            assign[(core, s)] = (b, list(range(next_chunk, next_chunk + take)))
            next_chunk += take
            rem -= take
    return assign


def _schedule(chunks):
    """Pick slot capacities (shared program structure) + assignment."""
    total = sum(chunks)
    lo = max(1, math.ceil(total / NCORES))
    for U in range(lo, 17):
        caps_opts = []
        for c0 in range(U, 0, -1):
            for c1 in range(min(c0, U - c0), -1, -1):
                c2 = U - c0 - c1
                if c2 < 0 or c2 > c1:
                    continue
                caps = tuple(c for c in (c0, c1, c2) if c > 0)
                caps_opts.append(caps)
        caps_opts.sort(key=lambda cs: (len(cs), max(cs)))
        for caps in caps_opts:
            asg = _greedy_assign(chunks, caps)
            if asg is not None:
                return caps, asg
    caps = (16,)
    asg = {(b, 0): (b, list(range(chunks[b]))) for b in range(B)}
    return caps, asg


# --------------------------------------------------------------------------
# device program (one NEFF shared by all 8 cores; structure = caps)
# --------------------------------------------------------------------------

def _build_program(caps):
    S = len(caps)
    U = sum(caps)
    nc = bacc.Bacc("TRN2", target_bir_lowering=False)

    # q's two halves live on the two partition halves so DMA uses all 16
    # SBUF ports; K^T is duplicated so either row-group can contract with it.
    # All DRAM layouts match the SBUF layouts so each DMA moves one long
    # contiguous run per partition (fewest descriptors).
    qts_d = nc.dram_tensor("qts", [S, 2 * D, QH], BF16, kind="ExternalInput")
    ktp_d = nc.dram_tensor("ktp", [2 * D, U, CHUNK], BF16, kind="ExternalInput")
    vp_d = nc.dram_tensor("vp", [CHUNK, U, VPW], BF16, kind="ExternalInput")
    mbb_d = nc.dram_tensor("mbb", [CHUNK, 2, U], F32, kind="ExternalInput")
    out_d = nc.dram_tensor("out", [S, D + 1, L], BF16, kind="ExternalOutput")

    # global unit order with slot bookkeeping
    slot_of = []
    idx_in_slot = []
    for s, c in enumerate(caps):
        for i in range(c):
            slot_of.append(s)
            idx_in_slot.append(i)

    with tile.TileContext(nc) as tc:
        with (
            tc.tile_pool(name="const", bufs=1) as const,
            tc.tile_pool(name="psS", bufs=2, space="PSUM") as psS_pool,
            tc.tile_pool(name="psO", bufs=2, space="PSUM") as psO_pool,
            tc.tile_pool(name="epool", bufs=5) as epool,
            tc.tile_pool(name="stage", bufs=3) as stage_pool,
        ):
            qts_sb = const.tile([2 * D, S, QH], BF16, tag="qts")
            ktp_sb = const.tile([2 * D, U, CHUNK], BF16, tag="ktp")
            vp_sb = const.tile([CHUNK, U, VPW], BF16, tag="vp")
            mbb_sb = const.tile([CHUNK, 2, U], F32, tag="mbb")
            warm_sb = const.tile([128, 512], BF16, tag="warm")

            # PE warm-up: dummy bf16 matmuls with no DMA dependency keep the
            # PE busy while inputs stream in, so the clock is ramped when the
            # first real matmul issues.  The memset is the very first gpsimd
            # instruction so warmups start as early as possible.
            nc.gpsimd.memset(warm_sb[:], 0.0)

            def emit_warm(n):
                for wi in range(n):
                    wps = psO_pool.tile([128, 512], F32, tag="psO")
                    nc.tensor.matmul(wps[:], warm_sb[:, :128], warm_sb[:], start=True, stop=True)

            emit_warm(N_WARM_PRE)

            # ---- input DMA: two HWDGE rings, ring-FIFO = priority order.
            # The first units' working set heads each ring; bulk follows and
            # naturally waits its turn, so the critical bytes never contend
            # for HBM bandwidth with the bulk.
            u3 = min(3, U)
            # scalar ring: K^T unit 0 (smallest, gates the first matmul),
            # K^T units 1-2, V'+biases units 0-2 (gates the first exp),
            # then the bulk
            nc.scalar.dma_start(ktp_sb[:, 0:1, :], ktp_d[:, 0:1, :])
            if u3 > 1:
                nc.scalar.dma_start(ktp_sb[:, 1:u3, :], ktp_d[:, 1:u3, :])
            nc.scalar.dma_start(vp_sb[:, 0:u3, :], vp_d[:, 0:u3, :])
            if U > u3:
                nc.scalar.dma_start(ktp_sb[:, u3:U, :], ktp_d[:, u3:U, :])
                nc.scalar.dma_start(vp_sb[:, u3:U, :], vp_d[:, u3:U, :])
            # sync ring: mask biases (small, hides under the warmup window),
            # then slot-0 Q by column halves, then remaining Q slots
            nc.sync.dma_start(mbb_sb[:], mbb_d[:, :, :])
            nc.sync.dma_start(qts_sb[:, 0, 0:512], qts_d[0, :, 0:512])
            nc.sync.dma_start(qts_sb[:, 0, 512:QH], qts_d[0, :, 512:QH])
            for s in range(1, S):
                nc.sync.dma_start(qts_sb[:, s, :], qts_d[s, :, :])

            # ---- software pipeline over the global unit list ----
            def emit_S(t):
                s = slot_of[t]
                psS_h = [
                    psS_pool.tile([CHUNK, QH], F32, tag="psS", name=f"psS_{t}_{hh}")
                    for hh in range(2)
                ]
                for h in range(2):
                    rows = slice(h * D, (h + 1) * D)
                    for j in range(QH // 512):
                        nc.tensor.matmul(
                            psS_h[h][:, j * 512 : (j + 1) * 512],
                            ktp_sb[rows, t, :],
                            qts_sb[rows, s, j * 512 : (j + 1) * 512],
                            start=True,
                            stop=True,
                        )
                return psS_h

            def emit_exp(t, psS_h):
                # h0 on ACT (table exp); h1 Schraudolph int16 fast-exp into
                # bf16 bits — on DVE, except act_schraud units where the
                # same fast-exp runs on ACT (Identity activation) to balance
                # the two engines' totals.  The exp METHOD stays consistent
                # per (batch, half): mixing table-exp and Schraudolph within
                # one softmax column breaks the bias cancellation in the
                # numerator/denominator ratio (measured 3.2e-2 vs 1.3e-2).
                e_h = []
                for h in range(2):
                    e = epool.tile([CHUNK, QH], BF16, tag=f"e{h}")
                    if h == 0:
                        nc.scalar.activation(
                            e[:],
                            psS_h[h][:],
                            mybir.ActivationFunctionType.Exp,
                            bias=mbb_sb[:, 0, t : t + 1],
                            scale=SCALE,
                        )
                    elif t in act_schraud:
                        nc.scalar.activation(
                            e[:].bitcast(I16),
                            psS_h[h][:],
                            mybir.ActivationFunctionType.Identity,
                            bias=mbb_sb[:, 1, t : t + 1],
                            scale=SCHRAUD_A,
                        )
                    else:
                        nc.vector.tensor_scalar(
                            e[:].bitcast(I16),
                            psS_h[h][:],
                            SCHRAUD_A,
                            mbb_sb[:, 1, t : t + 1],
                            mybir.AluOpType.mult,
                            mybir.AluOpType.add,
                        )
                    e_h.append(e)
                return e_h

            def emit_AV(t, psO_h, e_h):
                s = slot_of[t]
                first = idx_in_slot[t] == 0
                last = idx_in_slot[t] == caps[s] - 1
                for h in range(2):
                    for j in range(QH // 512):
                        nc.tensor.matmul(
                            psO_h[h][:, j * 512 : (j + 1) * 512],
                            vp_sb[:, t, 0 : D + 1],
                            e_h[h][:, j * 512 : (j + 1) * 512],
                            start=first,
                            stop=last,
                        )

            def emit_out(s, psO_h, last=False):
                # cast h0 on ACT, h1 on DVE, stream to DRAM on the two hwdge
                # rings.  Only the final slot casts in 512-col pieces
                # (subtile deps let piece j0 start as soon as the final AV j0
                # matmul lands) to shorten the kernel tail; piece-casting
                # every slot bursts the exp engines at slot boundaries and
                # stalls the PE.
                st0 = stage_pool.tile([D + 1, QH], BF16, tag="st0")
                st1 = stage_pool.tile([D + 1, QH], BF16, tag="st1")
                npc = QH // 512 if last else 1
                w = QH // npc
                for j in range(npc):
                    c = slice(j * w, (j + 1) * w)
                    nc.scalar.activation(st0[:, c], psO_h[0][:, c], mybir.ActivationFunctionType.Copy)
                    nc.sync.dma_start(out_d[s, :, j * w : (j + 1) * w], st0[:, c])
                    nc.vector.tensor_copy(st1[:, c], psO_h[1][:, c])
                    nc.scalar.dma_start(out_d[s, :, QH + j * w : QH + (j + 1) * w], st1[:, c])

            # units whose h1 Schraudolph runs on ACT instead of DVE (engine
            # load balance only — same method, so columns stay consistent)
            act_schraud = set()  # ACT Identity triggers a table switch (~3us)
                                 # mid-kernel — measured slower; keep all
                                 # Schraudolph on DVE

            slot_psO = {}  # slot -> psO tile pair, allocated at first AV

            def do_AV(pt, p_eh):
                ps = slot_of[pt]
                if idx_in_slot[pt] == 0:
                    slot_psO[ps] = [
                        psO_pool.tile([D + 1, QH], F32, tag="psO", name=f"psO_{ps}_{hh}")
                        for hh in range(2)
                    ]
                emit_AV(pt, slot_psO[ps], p_eh)
                if idx_in_slot[pt] == caps[ps] - 1:
                    emit_out(ps, slot_psO[ps], last=(pt == U - 1))

            pend = None  # (t, e_h) awaiting AV
            for t in range(U):
                psS_h = emit_S(t)
                if t == 1:
                    emit_warm(N_WARM_POST1)
                if pend is not None:
                    pt, p_eh = pend
                    do_AV(pt, p_eh)
                e_h = emit_exp(t, psS_h)
                pend = (t, e_h)
                if t == 0:
                    # fill the pipeline-fill bubble (unit 1's S waits on
                    # unit 0's exp) with more clock-warming dummies
                    emit_warm(N_WARM_POST)
            pt, p_eh = pend
            do_AV(pt, p_eh)
    nc.compile()
    return nc


# --------------------------------------------------------------------------
# host packing + gather
# --------------------------------------------------------------------------

def _pack_inputs(Q, K, V, valid_len, caps, asg):
    S = len(caps)
    U = sum(caps)
    slot_u0 = np.cumsum([0] + list(caps))[:-1]

    QT = np.ascontiguousarray(Q.transpose(0, 2, 1))  # [B, D, L]
    KT = np.ascontiguousarray(K.transpose(0, 2, 1))  # [B, D, L]

    bf16 = ml_dtypes.bfloat16
    in_maps = []
    for core in range(NCORES):
        qts = np.zeros((S, 2 * D, QH), bf16)
        ktp = np.zeros((2 * D, U, CHUNK), bf16)
        vp = np.zeros((CHUNK, U, VPW), bf16)
        # default biases for unassigned units: fully masked
        mbb = np.zeros((CHUNK, U, 2), np.float32)
        mbb[:, :, 0] = NEG
        mbb[:, :, 1] = -3.0e7
        for s in range(S):
            ent = asg.get((core, s))
            if ent is None:
                continue
            b, chunk_ids = ent
            qts[s, :D] = QT[b][:, :QH]
            qts[s, D:] = QT[b][:, QH:]
            for i, c in enumerate(chunk_ids):
                u = slot_u0[s] + i
                k0 = c * CHUNK
                ktp[:D, u] = KT[b][:, k0 : k0 + CHUNK]
                ktp[D:, u] = KT[b][:, k0 : k0 + CHUNK]
                vp[:, u, :D] = V[b][k0 : k0 + CHUNK]
                nvalid = int(min(max(valid_len[b] - k0, 0), CHUNK))
                vp[:nvalid, u, D] = 1.0
                mbb[:nvalid, u, 0] = 0.0
                mbb[:nvalid, u, 1] = SCHRAUD_B0
        mbb2 = np.ascontiguousarray(mbb.transpose(0, 2, 1))  # [CHUNK, 2, U]
        in_maps.append({"qts": qts, "ktp": ktp, "vp": vp, "mbb": mbb2})
    return in_maps


def _gather(results, caps, asg):
    acc = np.zeros((B, D + 1, L), np.float64)
    for core in range(NCORES):
        out = results[core]["out"]  # [S, D+1, L]
        for s in range(len(caps)):
            ent = asg.get((core, s))
            if ent is None:
                continue
            b, _ = ent
            acc[b] += np.asarray(out[s], dtype=np.float64)
    out = acc[:, :D, :] / acc[:, D : D + 1, :]
    return np.ascontiguousarray(out.transpose(0, 2, 1)).astype(np.float32)


_PROGRAM_CACHE = {}


def kernel(Q, K, V, valid_len, **kw):
    Q = np.asarray(Q, dtype=np.float32)
    K = np.asarray(K, dtype=np.float32)
    V = np.asarray(V, dtype=np.float32)
    vl = np.asarray(valid_len).astype(np.int64)

    chunks = [int(math.ceil(max(int(v), 1) / CHUNK)) for v in vl]
    caps, asg = _schedule(chunks)

    if caps not in _PROGRAM_CACHE:
        _PROGRAM_CACHE[caps] = _build_program(caps)
    nc = _PROGRAM_CACHE[caps]

    in_maps = _pack_inputs(Q, K, V, vl, caps, asg)
    res = run_bass_kernel_spmd(nc, in_maps, core_ids=list(range(NCORES)))
    return _gather(res.results, caps, asg)
